# revision 1
# baseline (speedup 1.0000x reference)
"""Causal self-attention (single head) on 8 Trainium2 NeuronCores.

Sharding: 8 cores = 4 batches x 2 query-tile parity sets. Core c handles
batch (c % 4). Cores 0-3 take query tiles t in {15,13,...,1} (128 rows
each), cores 4-7 take t in {14,12,...,0}. Attention iteration i=0..7 uses
a fixed causal extent E(i) = 16-2i k-tiles, so a single SPMD program
serves all cores; even-parity cores waste one fully-masked k-tile per
iteration.

Host passes x.T (plus the core's own query columns pre-gathered) and W.T
per core so the device never transposes inputs; operands are fp16 with
f32 PSUM accumulation. Softmax skips max-subtraction (scores/32 stay in a
safe exp range) and gets row sums free via the activation accum_out. All
operands stay SBUF-resident; Q.T is produced straight into SBUF.
"""

import sys

for _p in ("/opt/trn_rl_repo", "/root/.axon_site/_ro/trn_rl_repo"):
    if _p not in sys.path:
        sys.path.append(_p)

import numpy as np

import concourse.bass as bass  # noqa: F401
import concourse.mybir as mybir
import concourse.tile as tile
from concourse import bacc
from concourse.bass_utils import run_bass_kernel_spmd

F32 = mybir.dt.float32
F16 = mybir.dt.float16

BATCH, SEQ, D, P = 4, 2048, 1024, 1024
N_CORES = 8
QT = 128          # query tile rows
KTL = 128         # key tile
NBLK = 512        # matmul moving free dim
ND = D // 128     # 8 d-tiles
NP = P // 128     # 8 p-tiles
NKT = SEQ // KTL  # 16 k-tiles
NQT = 8           # q-tiles per core
SCALE = 1.0 / float(np.sqrt(P))
NEG = -1e9


def _extent(i):
    return 16 - 2 * i


def _chunks(width):
    out = []
    w = width
    while w >= NBLK:
        out.append(NBLK)
        w -= NBLK
    if w:
        assert w == 256, w
        out.append(256)
    return out


def build_program():
    nc = bacc.Bacc("TRN2", target_bir_lowering=False)

    xT = nc.dram_tensor("xT", [D, SEQ], F16, kind="ExternalInput")
    xn = nc.dram_tensor("xn", [SEQ, D], F16, kind="ExternalInput")
    xq_in = nc.dram_tensor("xqcols", [D, NQT * QT], F16, kind="ExternalInput")
    AT = nc.dram_tensor("AT", [D, D], F16, kind="ExternalInput")
    WvT = nc.dram_tensor("WvT", [D, P], F16, kind="ExternalInput")
    mask = nc.dram_tensor("mask", [QT, 256], F32, kind="ExternalInput")
    ident_in = nc.dram_tensor("ident", [128, 128], F16, kind="ExternalInput")
    out = nc.dram_tensor("out", [NQT * QT, P], F32, kind="ExternalOutput")

    # [128, dt, cols] views (partition dim first); full-row reads keep the
    # DMA's contiguous runs at row length (2-4KB), not a sliced 1KB.
    xT_r = xT.rearrange("(dt dp) s -> dp dt s", dp=128)
    xn_r = xn.rearrange("(kt kp) d -> kp kt d", kp=128)
    xq_r = xq_in.rearrange("(dt dp) q -> dp dt q", dp=128)
    at_r = AT.rearrange("(dt dp) d -> dp dt d", dp=128)
    wv_r = WvT.rearrange("(dt dp) p -> dp dt p", dp=128)

    with tile.TileContext(nc) as tc:
        with (
            tc.tile_pool(name="resident", bufs=1) as resident,
            tc.tile_pool(name="wrow", bufs=2) as wrow,
            tc.tile_pool(name="small", bufs=6) as small,
            tc.tile_pool(name="outp", bufs=2) as outp,
            tc.tile_pool(name="p0psum", bufs=3, space="PSUM") as p0psum,
            tc.tile_pool(name="zpsum", bufs=3, space="PSUM") as zpsum,
            tc.tile_pool(name="tpsum", bufs=2, space="PSUM") as tpsum,
        ):
            kt_sb = resident.tile([128, NP, SEQ], F16)    # G = A x^T [d, k]
            xn_all = resident.tile([128, NKT, D], F16)    # x natural [k, d]
            xq_all = resident.tile([128, ND, NQT * QT], F16)  # x.T q-cols
            xk_all = resident.tile([128, ND, SEQ], F16)   # x.T resident
            at_sb = resident.tile([128, ND, D], F16)      # A^T = Wk^T Wq
            wv_sb = resident.tile([128, ND, P], F16)
            mask_sb = resident.tile([QT, 256], F32)
            ident = resident.tile([128, 128], F16)
            cbias = resident.tile([QT, 1], F32)
            nc.vector.memset(cbias, -4.0)

            # startup loads: small constants, then per-d wq/xq pieces on the
            # sync queue (compute starts after the first pieces); bulk
            # tensors on the scalar queue, needed only ~40us in.
            # G-loop inputs by d-tile on the sync queue (full rows keep
            # 2-4KB DMA runs); everything else on the scalar queue.
            nc.sync.dma_start(out=ident, in_=ident_in[:, :])
            H = SEQ // 2
            for d in range(ND):
                nc.sync.dma_start(out=at_sb[:, d, :], in_=at_r[:, d, :])
                nc.sync.dma_start(
                    out=xk_all[:, d, 0:H], in_=xT_r[:, d, 0:H])
            for d in range(ND):
                nc.sync.dma_start(
                    out=xk_all[:, d, H:SEQ], in_=xT_r[:, d, H:SEQ])
            nc.scalar.dma_start(out=mask_sb, in_=mask[:, :])
            nc.scalar.dma_start(out=xq_all, in_=xq_r)
            nc.scalar.dma_start(
                out=xn_all[:, 0:NKT // 2, :], in_=xn_r[:, 0:NKT // 2, :])
            nc.scalar.dma_start(out=wv_sb, in_=wv_r)
            nc.scalar.dma_start(
                out=xn_all[:, NKT // 2:NKT, :], in_=xn_r[:, NKT // 2:NKT, :])

            # --- G = A x^T and V production ---
            for kb in range(SEQ // NBLK):
                for pt in range(NP):
                    ps = p0psum.tile([128, NBLK], F32, tag="p0")
                    for d in range(ND):
                        nc.tensor.matmul(
                            ps,
                            at_sb[:, d, pt * 128:(pt + 1) * 128],
                            xk_all[:, d, kb * NBLK:(kb + 1) * NBLK],
                            start=(d == 0),
                            stop=(d == ND - 1),
                        )
                    nc.scalar.copy(kt_sb[:, pt, kb * NBLK:(kb + 1) * NBLK], ps)

            # --- attention, smallest extent first (unlocks earliest) ---
            for i in (7, 6, 5, 4, 3, 2, 1, 0):
                ext = _extent(i)
                width = ext * KTL
                chunks = _chunks(width)

                s_ps = []
                off = 0
                for cw in chunks:
                    ps_full = p0psum.tile([QT, NBLK], F32, tag="p0")
                    ps = ps_full[:, :cw]
                    for pt in range(NP):
                        nc.tensor.matmul(
                            ps,
                            xq_all[:, pt, i * QT:(i + 1) * QT],
                            kt_sb[:, pt, off:off + cw],
                            start=(pt == 0),
                            stop=(pt == NP - 1),
                        )
                    s_ps.append((ps, off, cw))
                    off += cw

                # additive causal mask on the last 256 columns of the row
                last_ps, _, last_w = s_ps[-1]
                nc.vector.tensor_add(
                    last_ps[:, last_w - 256:last_w],
                    last_ps[:, last_w - 256:last_w],
                    mask_sb,
                )

                # exp((s + m) * scale) -> fp16 weights row; row sums free
                w_sb = wrow.tile([QT, width], F16, tag="w")
                lparts = small.tile([QT, len(chunks)], F32, tag="lp")
                for ci, (ps, off_c, cw) in enumerate(s_ps):
                    nc.scalar.activation(
                        w_sb[:, off_c:off_c + cw],
                        ps,
                        mybir.ActivationFunctionType.Exp,
                        scale=SCALE,
                        bias=cbias,
                        accum_out=lparts[:, ci:ci + 1],
                    )

                lsum = small.tile([QT, 1], F32, tag="ls")
                nc.vector.reduce_sum(lsum, lparts, axis=mybir.AxisListType.X)
                rl = small.tile([QT, 1], F32, tag="rl")
                nc.vector.reciprocal(rl, lsum)

                # U = W x  (transpose each weight block on PE)
                u0 = zpsum.tile([QT, NBLK], F32, tag="z")
                u1 = zpsum.tile([QT, NBLK], F32, tag="z")
                for kt in range(ext):
                    tp = tpsum.tile([128, 128], F16, tag="tp")
                    nc.tensor.transpose(
                        tp, w_sb[:, kt * 128:(kt + 1) * 128], ident)
                    wT = small.tile([128, 128], F16, tag="wT")
                    nc.vector.tensor_copy(wT, tp)
                    nc.tensor.matmul(
                        u0, wT, xn_all[:, kt, 0:NBLK],
                        start=(kt == 0), stop=(kt == ext - 1),
                    )
                    nc.tensor.matmul(
                        u1, wT, xn_all[:, kt, NBLK:D],
                        start=(kt == 0), stop=(kt == ext - 1),
                    )
                u_sb = wrow.tile([QT, D], F16, tag="u")
                nc.scalar.copy(u_sb[:, 0:NBLK], u0)
                nc.vector.tensor_copy(u_sb[:, NBLK:D], u1)

                # Z = U Wv^T  (U transposed per d-tile on PE)
                uT_sb = small.tile([128, ND, 128], F16, tag="uT")
                for dt in range(ND):
                    tpu = tpsum.tile([128, 128], F16, tag="tp")
                    nc.tensor.transpose(
                        tpu, u_sb[:, dt * 128:(dt + 1) * 128], ident)
                    nc.vector.tensor_copy(uT_sb[:, dt, :], tpu)
                z0 = zpsum.tile([QT, NBLK], F32, tag="z")
                z1 = zpsum.tile([QT, NBLK], F32, tag="z")
                for dt in range(ND):
                    nc.tensor.matmul(
                        z0, uT_sb[:, dt, :], wv_sb[:, dt, 0:NBLK],
                        start=(dt == 0), stop=(dt == ND - 1),
                    )
                    nc.tensor.matmul(
                        z1, uT_sb[:, dt, :], wv_sb[:, dt, NBLK:P],
                        start=(dt == 0), stop=(dt == ND - 1),
                    )

                o_sb = outp.tile([QT, P], F32, tag="o")
                nc.vector.tensor_scalar_mul(o_sb[:, 0:NBLK], z0, rl)
                nc.vector.tensor_scalar_mul(o_sb[:, NBLK:P], z1, rl)
                nc.sync.dma_start(out=out[i * QT:(i + 1) * QT, :], in_=o_sb)

    nc.compile()
    return nc


def _tiles_for_core(c):
    """Global 128-row query-tile indices, in program order i=0..7."""
    return [(15 - 2 * i) if c < 4 else (14 - 2 * i) for i in range(NQT)]


def _host_prep(inputs, Wq, Wk, Wv):
    x = np.asarray(inputs, dtype=np.float32)
    Wqf = np.asarray(Wq, dtype=np.float32)
    Wkf = np.asarray(Wk, dtype=np.float32)
    # scores = x (Wq^T Wk) x^T; device stationary wants the transpose
    ATm = np.ascontiguousarray((Wkf.T @ Wqf).astype(np.float16))
    WvT = np.ascontiguousarray(
        np.asarray(Wv, dtype=np.float32).T.astype(np.float16))

    qi = np.arange(QT)[:, None]
    ki = np.arange(128)[None, :]
    tri = np.where(qi >= ki, 0.0, NEG).astype(np.float32)
    mask_hi = np.concatenate([np.zeros((QT, 128), np.float32), tri], axis=1)
    mask_lo = np.concatenate(
        [tri, np.full((QT, 128), NEG, np.float32)], axis=1)

    in_maps = []
    xT_cache = {}
    for c in range(N_CORES):
        b = c % 4
        if b not in xT_cache:
            xT_cache[b] = np.ascontiguousarray(x[b].T.astype(np.float16))
        xTb = xT_cache[b]
        cols = np.concatenate(
            [xTb[:, t * QT:(t + 1) * QT] for t in _tiles_for_core(c)], axis=1)
        in_maps.append({
            "xT": xTb,
            "xn": np.ascontiguousarray(x[b].astype(np.float16)),
            "xqcols": np.ascontiguousarray(cols),
            "AT": ATm,
            "WvT": WvT,
            "mask": mask_hi if c < 4 else mask_lo,
            "ident": np.eye(128, dtype=np.float16),
        })
    return in_maps


def _host_gather(results):
    Z = np.empty((BATCH, SEQ, P), dtype=np.float32)
    for c in range(N_CORES):
        b = c % 4
        o = results[c]["out"]
        for i, t in enumerate(_tiles_for_core(c)):
            Z[b, t * QT:(t + 1) * QT, :] = o[i * QT:(i + 1) * QT, :]
    return Z


_NC_CACHE = None


def kernel(inputs, Wq, Wk, Wv):
    global _NC_CACHE
    if _NC_CACHE is None:
        _NC_CACHE = build_program()
    in_maps = _host_prep(inputs, Wq, Wk, Wv)
    # The first execution after a fresh compile occasionally hits a
    # transient NRT_EXEC_UNIT_UNRECOVERABLE; a retry reliably succeeds.
    last_err = None
    Z = None
    for _ in range(3):
        try:
            res = run_bass_kernel_spmd(
                _NC_CACHE, in_maps, list(range(N_CORES)))
            Z = _host_gather(res.results)
            if np.isfinite(Z).all():
                return Z
        except Exception as e:  # noqa: BLE001
            last_err = e
    if Z is not None:
        return Z
    raise last_err



# revision 3
# speedup vs baseline: 1.3229x; 1.3229x over previous
"""Causal self-attention (single head) on 8 Trainium2 NeuronCores.

Sharding: 8 cores = 4 batches x 2 query-tile parity sets. Core c handles
batch (c % 4). Cores 0-3 take query tiles t in {15,13,...,1} (128 rows
each), cores 4-7 take t in {14,12,...,0}. Attention iteration i uses a
fixed causal extent E(i) = 16-2i k-tiles, so a single SPMD program
serves all cores; even-parity cores waste one fully-masked k-tile per
iteration.

Math: scores = x Wq^T Wk x^T = (x A) x^T with A = Wq^T Wk folded on the
host. The kernel computes Q'^T = A^T-blocks x x_q^T (64 units over the
core's own 1024 q rows) instead of the key-side G = A x^T (128 units
over all 2048 keys) -- halving the projection phase. All transposes
(softmax weights W and the U = W x intermediate) go through the DMA
xbar (dma_start_transpose) instead of the PE array, freeing ~35us of
tensor-engine time. Startup interleaves A^T/x_q loads per 128-row slice
with an et-outer 8-bank PSUM accumulation so the PE starts ~1.5us in.
Iterations are software-pipelined: step j emits scores(j), U(j-1),
Z(j-2), hiding the exp->transpose->matmul latency chains under other
PE work. Softmax skips max-subtraction (scores/32 stay in a safe exp
range) and row sums come free via the activation accum_out.
"""

import sys

for _p in ("/opt/trn_rl_repo", "/root/.axon_site/_ro/trn_rl_repo"):
    if _p not in sys.path:
        sys.path.append(_p)

import numpy as np

import concourse.bass as bass  # noqa: F401
import concourse.mybir as mybir
import concourse.tile as tile
from concourse import bacc
from concourse.bass_utils import run_bass_kernel_spmd

F32 = mybir.dt.float32
F16 = mybir.dt.float16

BATCH, SEQ, D, P = 4, 2048, 1024, 1024
N_CORES = 8
QT = 128          # query tile rows
KTL = 128         # key tile
NBLK = 512        # matmul moving free dim
ND = D // 128     # 8 d-tiles
NKT = SEQ // KTL  # 16 k-tiles
NQT = 8           # q-tiles per core
SCALE = 1.0 / float(np.sqrt(P))
NEG = -1e9

# processing order over program q-tile index i (extent = 16-2i): start
# small (xk/xn prefixes arrive first), grow to the full extent, and end
# on the smallest tile so the drain tail is short.
ORDER = (6, 5, 4, 3, 2, 1, 0, 7)


def _extent(i):
    return 16 - 2 * i


def _chunks(width):
    out = []
    w = width
    while w >= NBLK:
        out.append(NBLK)
        w -= NBLK
    if w:
        assert w == 256, w
        out.append(256)
    return out


def build_program():
    nc = bacc.Bacc("TRN2", target_bir_lowering=False)

    xT = nc.dram_tensor("xT", [D, SEQ], F16, kind="ExternalInput")
    xn = nc.dram_tensor("xn", [SEQ, D], F16, kind="ExternalInput")
    xq_in = nc.dram_tensor("xqcols", [D, NQT * QT], F16, kind="ExternalInput")
    AT = nc.dram_tensor("AT", [D, D], F16, kind="ExternalInput")
    WvT = nc.dram_tensor("WvT", [D, P], F16, kind="ExternalInput")
    mask = nc.dram_tensor("mask", [QT, 256], F32, kind="ExternalInput")
    out = nc.dram_tensor("out", [NQT * QT, P], F32, kind="ExternalOutput")

    # [128, et/dt/kt, cols] views (partition dim first); full-row reads
    # keep the DMA's contiguous runs long.
    xT_r = xT.rearrange("(dt dp) s -> dp dt s", dp=128)
    xn_r = xn.rearrange("(kt kp) d -> kp kt d", kp=128)
    xq_r = xq_in.rearrange("(et ep) q -> ep et q", ep=128)
    at_r = AT.rearrange("(et ep) d -> ep et d", ep=128)
    wv_r = WvT.rearrange("(dt dp) p -> dp dt p", dp=128)

    with tile.TileContext(nc) as tc:
        with (
            tc.tile_pool(name="resident", bufs=1) as resident,
            tc.tile_pool(name="wrow", bufs=2) as wrow,
            tc.tile_pool(name="tpool", bufs=2) as tpool,
            tc.tile_pool(name="small", bufs=6) as small,
            tc.tile_pool(name="outp", bufs=2) as outp,
        ):
            at_sb = resident.tile([128, ND, D], F16)      # A^T = Wk^T Wq
            xq_all = resident.tile([128, ND, NQT * QT], F16)  # x.T q-cols
            qT_sb = resident.tile([128, ND, NQT * QT], F16)   # Q'^T [d, q]
            xk_all = resident.tile([128, ND, SEQ], F16)   # x.T resident
            xn_all = resident.tile([128, NKT, D], F16)    # x natural [k, d]
            wv_sb = resident.tile([128, ND, P], F16)
            mask_sb = resident.tile([QT, 256], F32)
            cbias = resident.tile([QT, 1], F32)
            nc.vector.memset(cbias, -4.0)

            # startup loads: A^T rows + the x_q columns for the first Q'T
            # chunk interleaved per 128-row slice on the sync queue so the
            # first matmul can issue ~1.5us in; bulk tensors (xk/xn
            # quarters, wv) staggered on the scalar queue in the order the
            # attention iterations consume them.
            H2 = NQT * QT // 2  # 512
            for et in range(ND):
                nc.sync.dma_start(out=at_sb[:, et, :], in_=at_r[:, et, :])
                nc.sync.dma_start(
                    out=xq_all[:, et, H2:], in_=xq_r[:, et, H2:])
            for et in range(ND):
                nc.sync.dma_start(
                    out=xq_all[:, et, 0:H2], in_=xq_r[:, et, 0:H2])

            nc.scalar.dma_start(out=mask_sb, in_=mask[:, :])
            for q in range(4):
                c0, c1 = q * 512, (q + 1) * 512
                nc.scalar.dma_start(
                    out=xk_all[:, :, c0:c1], in_=xT_r[:, :, c0:c1])
                nc.scalar.dma_start(
                    out=xn_all[:, 4 * q:4 * (q + 1), :],
                    in_=xn_r[:, 4 * q:4 * (q + 1), :])
                if q == 0:
                    nc.scalar.dma_start(out=wv_sb, in_=wv_r)

            # --- Q'^T = A'^T-blocks x x_q (accumulate over et in 8 psum
            # banks so matmuls start as soon as the first rows land) ---
            with tc.tile_pool(name="p1ps", bufs=8, space="PSUM") as p1ps:
                for c0 in (H2, 0):  # high q-cols first (ORDER starts there)
                    ps = []
                    for dt in range(ND):
                        p1t = p1ps.tile([128, NBLK], F32, tag="p1")
                        ps.append(p1t)
                    for et in range(ND):
                        for dt in range(ND):
                            nc.tensor.matmul(
                                ps[dt],
                                at_sb[:, et, dt * 128:(dt + 1) * 128],
                                xq_all[:, et, c0:c0 + H2],
                                start=(et == 0),
                                stop=(et == ND - 1),
                            )
                    for dt in range(ND):
                        cp = (nc.scalar.copy if dt % 2 == 0
                              else nc.vector.tensor_copy)
                        cp(qT_sb[:, dt, c0:c0 + H2], ps[dt])

            # --- attention, software-pipelined: scores(j), U(j-1), Z(j-2)
            with (
                tc.tile_pool(name="sps", bufs=4, space="PSUM") as sps,
                tc.tile_pool(name="uzps", bufs=4, space="PSUM") as uzps,
            ):
                state = {}

                def emit_scores(i):
                    ext = _extent(i)
                    width = ext * KTL
                    s_list = []
                    off = 0
                    for cw in _chunks(width):
                        s_ps = sps.tile([QT, NBLK], F32, tag="s")
                        psv = s_ps[:, :cw]
                        for dt in range(ND):
                            nc.tensor.matmul(
                                psv,
                                qT_sb[:, dt, i * QT:(i + 1) * QT],
                                xk_all[:, dt, off:off + cw],
                                start=(dt == 0),
                                stop=(dt == ND - 1),
                            )
                        s_list.append((psv, off, cw))
                        off += cw

                    # additive causal mask on the last 256 columns
                    last_ps, _, cw_l = s_list[-1]
                    nc.vector.tensor_add(
                        last_ps[:, cw_l - 256:cw_l],
                        last_ps[:, cw_l - 256:cw_l],
                        mask_sb,
                    )

                    # exp((s + m) * scale - 4) -> fp16 weights row; row
                    # sums free via accum_out
                    w_sb = wrow.tile([QT, SEQ], F16, tag="w")
                    lparts = small.tile([QT, 4], F32, tag="lp")
                    for ci, (psv, off_c, cw) in enumerate(s_list):
                        nc.scalar.activation(
                            w_sb[:, off_c:off_c + cw],
                            psv,
                            mybir.ActivationFunctionType.Exp,
                            scale=SCALE,
                            bias=cbias,
                            accum_out=lparts[:, ci:ci + 1],
                        )
                    lsum = small.tile([QT, 1], F32, tag="ls")
                    nc.vector.reduce_sum(
                        lsum, lparts[:, :len(s_list)],
                        axis=mybir.AxisListType.X)
                    rl = small.tile([QT, 1], F32, tag="rl")
                    nc.vector.reciprocal(rl, lsum)

                    # blocked transpose W -> W^T tiles via the DMA xbar
                    wT = tpool.tile([128, NKT, 128], F16, tag="wT")
                    nc.sync.dma_start_transpose(
                        out=wT[:, 0:ext, :], in_=w_sb[:, 0:width])
                    state[i] = dict(ext=ext, wT=wT, rl=rl)

                def emit_U(i):
                    st = state[i]
                    ext = st["ext"]
                    wT = st["wT"]
                    u0 = uzps.tile([QT, NBLK], F32, tag="uz")
                    u1 = uzps.tile([QT, NBLK], F32, tag="uz")
                    for kt in range(ext):
                        nc.tensor.matmul(
                            u0, wT[:, kt, :], xn_all[:, kt, 0:NBLK],
                            start=(kt == 0), stop=(kt == ext - 1),
                        )
                        nc.tensor.matmul(
                            u1, wT[:, kt, :], xn_all[:, kt, NBLK:D],
                            start=(kt == 0), stop=(kt == ext - 1),
                        )
                    u_sb = wrow.tile([QT, D], F16, tag="u")
                    nc.scalar.copy(u_sb[:, 0:NBLK], u0)
                    nc.vector.tensor_copy(u_sb[:, NBLK:D], u1)
                    uT = tpool.tile([128, ND, 128], F16, tag="uT")
                    nc.sync.dma_start_transpose(out=uT, in_=u_sb)
                    st["uT"] = uT

                def emit_Z(i):
                    st = state.pop(i)
                    uT = st["uT"]
                    z0 = uzps.tile([QT, NBLK], F32, tag="uz")
                    z1 = uzps.tile([QT, NBLK], F32, tag="uz")
                    for dt in range(ND):
                        nc.tensor.matmul(
                            z0, uT[:, dt, :], wv_sb[:, dt, 0:NBLK],
                            start=(dt == 0), stop=(dt == ND - 1),
                        )
                        nc.tensor.matmul(
                            z1, uT[:, dt, :], wv_sb[:, dt, NBLK:P],
                            start=(dt == 0), stop=(dt == ND - 1),
                        )
                    o_sb = outp.tile([QT, P], F32, tag="o")
                    nc.vector.tensor_scalar_mul(o_sb[:, 0:NBLK], z0, st["rl"])
                    nc.vector.tensor_scalar_mul(o_sb[:, NBLK:P], z1, st["rl"])
                    nc.sync.dma_start(
                        out=out[i * QT:(i + 1) * QT, :], in_=o_sb)

                for j in range(NQT):
                    emit_scores(ORDER[j])
                    if j >= 1:
                        emit_U(ORDER[j - 1])
                    if j >= 2:
                        emit_Z(ORDER[j - 2])
                emit_U(ORDER[NQT - 1])
                emit_Z(ORDER[NQT - 2])
                emit_Z(ORDER[NQT - 1])

    nc.compile()
    return nc


def _tiles_for_core(c):
    """Global 128-row query-tile indices, indexed by program i=0..7."""
    return [(15 - 2 * i) if c < 4 else (14 - 2 * i) for i in range(NQT)]


def _host_prep(inputs, Wq, Wk, Wv):
    x = np.asarray(inputs, dtype=np.float32)
    Wqf = np.asarray(Wq, dtype=np.float32)
    Wkf = np.asarray(Wk, dtype=np.float32)
    # scores = x A' x^T with A' = Wq^T Wk; the Q'^T matmul contracts over
    # A's rows (lhsT[e, d] = A'[e, d]), so A' itself is the stationary.
    ATm = np.ascontiguousarray((Wqf.T @ Wkf).astype(np.float16))
    WvT = np.ascontiguousarray(
        np.asarray(Wv, dtype=np.float32).T.astype(np.float16))

    qi = np.arange(QT)[:, None]
    ki = np.arange(128)[None, :]
    tri = np.where(qi >= ki, 0.0, NEG).astype(np.float32)
    mask_hi = np.concatenate([np.zeros((QT, 128), np.float32), tri], axis=1)
    mask_lo = np.concatenate(
        [tri, np.full((QT, 128), NEG, np.float32)], axis=1)

    in_maps = []
    xT_cache = {}
    for c in range(N_CORES):
        b = c % 4
        if b not in xT_cache:
            xT_cache[b] = np.ascontiguousarray(x[b].T.astype(np.float16))
        xTb = xT_cache[b]
        cols = np.concatenate(
            [xTb[:, t * QT:(t + 1) * QT] for t in _tiles_for_core(c)], axis=1)
        in_maps.append({
            "xT": xTb,
            "xn": np.ascontiguousarray(x[b].astype(np.float16)),
            "xqcols": np.ascontiguousarray(cols),
            "AT": ATm,
            "WvT": WvT,
            "mask": mask_hi if c < 4 else mask_lo,
        })
    return in_maps


def _host_gather(results):
    Z = np.empty((BATCH, SEQ, P), dtype=np.float32)
    for c in range(N_CORES):
        b = c % 4
        o = results[c]["out"]
        for i, t in enumerate(_tiles_for_core(c)):
            Z[b, t * QT:(t + 1) * QT, :] = o[i * QT:(i + 1) * QT, :]
    return Z


_NC_CACHE = None


def kernel(inputs, Wq, Wk, Wv):
    global _NC_CACHE
    if _NC_CACHE is None:
        _NC_CACHE = build_program()
    in_maps = _host_prep(inputs, Wq, Wk, Wv)
    # The first execution after a fresh compile occasionally hits a
    # transient NRT_EXEC_UNIT_UNRECOVERABLE; a retry reliably succeeds.
    last_err = None
    Z = None
    for _ in range(3):
        try:
            res = run_bass_kernel_spmd(
                _NC_CACHE, in_maps, list(range(N_CORES)))
            Z = _host_gather(res.results)
            if np.isfinite(Z).all():
                return Z
        except Exception as e:  # noqa: BLE001
            last_err = e
    if Z is not None:
        return Z
    raise last_err
